# revision 1
# baseline (speedup 1.0000x reference)
"""Trainium2 Bass kernel for the HOI relation model.

Pipeline per core (2 images each, 8 cores data-parallel over batch):
  1. ROI mean pooling: pooled[d,c] = (1/area_d) * sum_hw mask[d,hw] * F[hw,c]
     computed as 32 K-chunk matmuls (mask stationary [128,32], features
     moving [128,768] in two N=384 halves), bf16 operands, f32 PSUM.
  2. PE-transpose pooled [32,768] -> pooledT [768,32] (6 transposes).
  3. Layer 1 factorized: relu(pair(h,o) @ w1 + b1) = relu(A(h) + B(o) + b1)
     where A = w1[:768].T @ h, B = w1[768:].T @ o  -- the 8x24 pair
     expansion happens AFTER the matmul (broadcast add on DVE).
  4. Layers 2, 3 as plain matmuls on the 384 pair rows (transposed layout).

Host does only O(B*D) prep: box->mask rasterization, score argsort
(baked into mask column order), 1/area, dtype casts, shard/gather.
"""

import numpy as np
import ml_dtypes

import concourse.bass as bass
import concourse.mybir as mybir
import concourse.tile as tile
from concourse import bacc
from concourse.bass_utils import run_bass_kernel_spmd
from concourse.masks import make_identity

N_CORES = 8
B, D, C = 16, 32, 768
NH, NO = 8, 24
NPAIR = NH * NO              # 192 pairs per image
GRID = 64                    # feature grid (896 / 14)
KPIX = GRID * GRID           # 4096 pixels per image
BL = B // N_CORES            # 2 images per core
KCH = KPIX // 128            # 32 K-chunks per image
CG = 4                       # K-chunks per DMA tile
H1, H2, H3 = 512, 256, 117
M = BL * NPAIR               # 384 pair rows per core

F32 = mybir.dt.float32
BF16 = mybir.dt.bfloat16
BF = ml_dtypes.bfloat16

_PROGRAM = None


def _build_program():
    nc = bacc.Bacc("TRN2", target_bir_lowering=False, debug=False,
                   num_devices=N_CORES)
    feat = nc.declare_dram_parameter("feat", [BL, KPIX, C], BF16, isOutput=False)
    maskT = nc.declare_dram_parameter("maskT", [BL, KPIX, D], BF16, isOutput=False)
    inva = nc.declare_dram_parameter("inva", [BL, D], F32, isOutput=False)
    w1 = nc.declare_dram_parameter("w1", [2 * C, H1], BF16, isOutput=False)
    b1 = nc.declare_dram_parameter("b1", [H1], F32, isOutput=False)
    w2 = nc.declare_dram_parameter("w2", [H1, H2], BF16, isOutput=False)
    b2 = nc.declare_dram_parameter("b2", [H2], F32, isOutput=False)
    w3 = nc.declare_dram_parameter("w3", [H2, H3], BF16, isOutput=False)
    b3 = nc.declare_dram_parameter("b3", [H3], F32, isOutput=False)
    out = nc.declare_dram_parameter("out", [M, H3], F32, isOutput=True)

    add = mybir.AluOpType.add
    amax = mybir.AluOpType.max

    with tile.TileContext(nc) as tc:
        with (
            tc.tile_pool(name="singles", bufs=1) as singles,
            tc.tile_pool(name="featp", bufs=6) as featp,
            tc.tile_pool(name="maskp", bufs=6) as maskp,
            tc.tile_pool(name="work", bufs=1) as work,
            tc.tile_pool(name="tmp", bufs=3) as tmpp,
            tc.tile_pool(name="pps", bufs=1, space="PSUM") as pps,
            tc.tile_pool(name="mps", bufs=4, space="PSUM") as mps,
        ):
            # ---- one-time constant loads ----
            ident = singles.tile([32, 32], BF16, tag="ident")
            make_identity(nc, ident)
            w1_sb = singles.tile([128, 12, H1], BF16, tag="w1")
            nc.sync.dma_start(out=w1_sb, in_=w1[:, :].rearrange("(kc p) n -> p kc n", p=128))
            w2_sb = singles.tile([128, 4, H2], BF16, tag="w2")
            nc.sync.dma_start(out=w2_sb, in_=w2[:, :].rearrange("(kc p) n -> p kc n", p=128))
            w3_sb = singles.tile([128, 2, H3], BF16, tag="w3")
            nc.sync.dma_start(out=w3_sb, in_=w3[:, :].rearrange("(kc p) n -> p kc n", p=128))
            b1_sb = singles.tile([128, 4], F32, tag="b1")
            nc.sync.dma_start(out=b1_sb, in_=b1[:].rearrange("(mc p) -> p mc", p=128))
            b2_sb = singles.tile([128, 2], F32, tag="b2")
            nc.sync.dma_start(out=b2_sb, in_=b2[:].rearrange("(mc p) -> p mc", p=128))
            b3_sb = singles.tile([128, H3], F32, tag="b3")
            b3_bcast = bass.AP(tensor=b3[:].tensor, offset=b3[:].offset,
                               ap=[[0, 128], [1, H3]])
            nc.sync.dma_start(out=b3_sb, in_=b3_bcast)
            inva_sb = singles.tile([D, BL], F32, tag="inva")
            nc.sync.dma_start(out=inva_sb, in_=inva[:, :].rearrange("b d -> d b"))

            # persistent activations
            pooledT = work.tile([128, BL, 6, D], BF16, tag="pooledT")
            x1T = work.tile([128, 4, M], BF16, tag="x1T")
            x2T = work.tile([128, 2, M], BF16, tag="x2T")

            # ---- pooling + transpose per image ----
            for img in range(BL):
                ps_a = pps.tile([D, 384], F32, tag=f"pp{img}a")
                ps_b = pps.tile([D, 384], F32, tag=f"pp{img}b")
                for g in range(KCH // CG):
                    f_sb = featp.tile([128, CG, C], BF16, tag="f")
                    nc.sync.dma_start(
                        out=f_sb,
                        in_=feat[img, g * CG * 128:(g + 1) * CG * 128, :]
                        .rearrange("(gc p) c -> p gc c", p=128))
                    m_sb = maskp.tile([128, CG, D], BF16, tag="m")
                    nc.sync.dma_start(
                        out=m_sb,
                        in_=maskT[img, g * CG * 128:(g + 1) * CG * 128, :]
                        .rearrange("(gc p) d -> p gc d", p=128))
                    for gc in range(CG):
                        kk = g * CG + gc
                        nc.tensor.matmul(ps_a, m_sb[:, gc, :], f_sb[:, gc, 0:384],
                                         start=(kk == 0), stop=(kk == KCH - 1))
                        nc.tensor.matmul(ps_b, m_sb[:, gc, :], f_sb[:, gc, 384:768],
                                         start=(kk == 0), stop=(kk == KCH - 1))
                # scale by 1/area, cast to bf16
                pooled = tmpp.tile([D, C], BF16, tag="pooled")
                nc.vector.tensor_scalar_mul(pooled[:, 0:384], ps_a, inva_sb[:, img:img + 1])
                nc.vector.tensor_scalar_mul(pooled[:, 384:768], ps_b, inva_sb[:, img:img + 1])
                # transpose to [C, D] in 6 chunks of 128 channels
                for cc in range(6):
                    ps_t = mps.tile([128, D], BF16, tag="mm")
                    nc.tensor.transpose(ps_t, pooled[:, cc * 128:(cc + 1) * 128], ident)
                    nc.vector.tensor_copy(pooledT[:, img, cc, :], ps_t)

            # ---- layer 1 (factorized over pairs) ----
            for mc in range(4):
                ps_ab = mps.tile([128, BL, D], F32, tag="mm")
                for kc in range(6):
                    nc.tensor.matmul(ps_ab[:, :, 0:NH],
                                     w1_sb[:, kc, mc * 128:(mc + 1) * 128],
                                     pooledT[:, :, kc, 0:NH],
                                     start=(kc == 0), stop=(kc == 5))
                for kc in range(6):
                    nc.tensor.matmul(ps_ab[:, :, NH:D],
                                     w1_sb[:, 6 + kc, mc * 128:(mc + 1) * 128],
                                     pooledT[:, :, kc, NH:D],
                                     start=(kc == 0), stop=(kc == 5))
                ab_sb = tmpp.tile([128, BL, D], F32, tag="ab")
                nc.vector.tensor_copy(ab_sb, ps_ab)
                for img in range(BL):
                    pre = tmpp.tile([128, NH, NO], F32, tag="pre")
                    a_bc = ab_sb[:, img, 0:NH][:, :, None].broadcast_to([128, NH, NO])
                    b_bc = ab_sb[:, img, NH:D][:, None, :].broadcast_to([128, NH, NO])
                    # pre = (A + b1) + B
                    nc.vector.scalar_tensor_tensor(pre, a_bc, b1_sb[:, mc:mc + 1],
                                                   b_bc, op0=add, op1=add)
                    dst = x1T[:, mc, img * NPAIR:(img + 1) * NPAIR] \
                        .rearrange("p (i j) -> p i j", i=NH)
                    nc.vector.tensor_scalar_max(dst, pre, 0.0)

            # ---- layer 2 ----
            for m2 in range(2):
                ps2 = mps.tile([128, M], F32, tag="mm")
                for kc in range(4):
                    nc.tensor.matmul(ps2, w2_sb[:, kc, m2 * 128:(m2 + 1) * 128],
                                     x1T[:, kc, :], start=(kc == 0), stop=(kc == 3))
                nc.vector.tensor_scalar(x2T[:, m2, :], ps2, b2_sb[:, m2:m2 + 1], 0.0,
                                        op0=add, op1=amax)

            # ---- layer 3 + bias + store ----
            for m3 in range(3):
                ps3 = mps.tile([128, H3], F32, tag="mm")
                for kc in range(2):
                    nc.tensor.matmul(ps3, x2T[:, kc, m3 * 128:(m3 + 1) * 128],
                                     w3_sb[:, kc, :], start=(kc == 0), stop=(kc == 1))
                o_sb = tmpp.tile([128, H3], F32, tag="osb")
                nc.vector.tensor_tensor(o_sb, ps3, b3_sb, op=add)
                nc.sync.dma_start(out=out[m3 * 128:(m3 + 1) * 128, :], in_=o_sb)
    nc.compile()
    return nc


def _get_program():
    global _PROGRAM
    if _PROGRAM is None:
        _PROGRAM = _build_program()
    return _PROGRAM


def _preprocess(boxes, scores):
    """Rasterize boxes to 0/1 masks with detection columns in sorted order."""
    cx, cy, bw, bh = boxes[..., 0], boxes[..., 1], boxes[..., 2], boxes[..., 3]
    x1 = np.floor((cx - bw / 2) * GRID).astype(np.int64)
    y1 = np.floor((cy - bh / 2) * GRID).astype(np.int64)
    x2 = np.floor((cx + bw / 2) * GRID).astype(np.int64)
    y2 = np.floor((cy + bh / 2) * GRID).astype(np.int64)
    hidx = np.argsort(-scores[:, :NH], axis=1, kind="stable")
    oidx = np.argsort(-scores[:, NH:], axis=1, kind="stable") + NH
    perm = np.concatenate([hidx, oidx], axis=1)                     # [B, D]
    g = np.arange(GRID)
    rows = (g[None, None, :] >= y1[..., None]) & (g[None, None, :] < y2[..., None])
    cols = (g[None, None, :] >= x1[..., None]) & (g[None, None, :] < x2[..., None])
    rows = np.take_along_axis(rows, perm[..., None], axis=1)        # [B, D, 64]
    cols = np.take_along_axis(cols, perm[..., None], axis=1)
    area = rows.sum(-1) * cols.sum(-1)                              # [B, D]
    mask = rows[:, :, :, None] & cols[:, :, None, :]                # [B, D, 64, 64]
    maskT = np.ascontiguousarray(
        mask.reshape(mask.shape[0], D, KPIX).transpose(0, 2, 1)).astype(BF)
    return maskT, (1.0 / area).astype(np.float32)


def _run(in_maps, trace=False, **kw):
    nc = _get_program()
    return run_bass_kernel_spmd(nc, in_maps, core_ids=list(range(N_CORES)),
                                trace=trace, **kw)


def _make_in_maps(features, boxes, scores, w1, b1, w2, b2, w3, b3):
    features = np.asarray(features, np.float32)
    maskT, inva = _preprocess(np.asarray(boxes, np.float32),
                              np.asarray(scores, np.float32))
    featb = np.ascontiguousarray(features.reshape(B, KPIX, C)).astype(BF)
    w1b = np.asarray(w1, np.float32).astype(BF)
    w2b = np.asarray(w2, np.float32).astype(BF)
    w3b = np.asarray(w3, np.float32).astype(BF)
    b1f = np.asarray(b1, np.float32)
    b2f = np.asarray(b2, np.float32)
    b3f = np.asarray(b3, np.float32)
    in_maps = []
    for c in range(N_CORES):
        s = slice(c * BL, (c + 1) * BL)
        in_maps.append({
            "feat": np.ascontiguousarray(featb[s]),
            "maskT": np.ascontiguousarray(maskT[s]),
            "inva": np.ascontiguousarray(inva[s]),
            "w1": w1b, "b1": b1f, "w2": w2b, "b2": b2f, "w3": w3b, "b3": b3f,
        })
    return in_maps


def kernel(features, boxes, scores, w1, b1, w2, b2, w3, b3, labels):
    in_maps = _make_in_maps(features, boxes, scores, w1, b1, w2, b2, w3, b3)
    res = _run(in_maps, trace=False)
    out = np.concatenate([r["out"].reshape(BL, NPAIR, H3) for r in res.results],
                         axis=0)
    return np.ascontiguousarray(out.astype(np.float32))



# revision 9
# speedup vs baseline: 1.5815x; 1.5815x over previous
"""Trainium2 Bass kernel for the HOI relation model (fp8 DoubleRow version).

Per core (2 images, 8 cores data-parallel over batch):
  1. Features are cropped on host to the 52x52 window that provably contains
     every box (setup_inputs: centers in [0.25,0.75], sizes in [0.08,0.30]
     => pixel corners in [6,58)), quantized to fp8e4m3 with 2D error-feedback
     (rounding residuals carried right/down so box-sums stay accurate), and
     laid out partition-major so every DMA line is contiguous.
  2. ROI mean pooling: DoubleRow fp8 matmuls (2 K-rows/cycle), mask
     stationary [128,2,32], features moving [128,2,256] in 3 C-chunks of 256.
     PSUM scaled by 1/area (f32) -> bf16 pooled -> PE-transpose -> pooledT.
  3. Layer 1 factorized + selector-matmul pair expansion:
     psA[8,512]  = sum_kc h_kc.T @ w1[kc]      (persons, w1 rows 0:768)
     psB[24,512] = sum_kc o_kc.T @ w1[6+kc]    (objects, w1 rows 768:1536)
     accumulated chunk-by-chunk as pooling finishes each kc, then
     pre[mc,pair] = ABt.T @ sel where sel[k,(i,j)] = [k==i] + [k==32+j]
     (A at partitions 0:8, B at 32:56 -- DVE writes need 32-aligned bases).
     ReLU+bias on the Scalar engine.
  4. Layers 2, 3 as plain matmuls on each image's 192 pair rows.

Host does box rasterization, score argsort (baked into mask column order),
1/area, fp8/bf16 casts and layout; all reference math runs on device.
"""

import numpy as np
import ml_dtypes

import concourse.bass as bass
import concourse.mybir as mybir
import concourse.tile as tile
from concourse import bacc
from concourse.bass_utils import run_bass_kernel_spmd
from concourse.masks import make_identity

N_CORES = 8
B, D, C = 16, 32, 768
NH, NO = 8, 24
NPAIR = NH * NO              # 192 pairs per image
GRID = 64                    # feature grid (896 / 14)
WIN = 52                     # crop window (boxes live in [6,58) for setup_inputs)
KPIX = 2816                  # padded pixels per image = 22*128
NKP = 11                     # DoubleRow K-pairs (2816 / 256)
NCC = 3                      # C-chunks
CCW = 256                    # channels per chunk
BL = B // N_CORES            # 2 images per core
H1, H2, H3 = 512, 256, 117
M = BL * NPAIR               # 384 pair rows per core

F32 = mybir.dt.float32
BF16 = mybir.dt.bfloat16
FP8 = mybir.dt.float8e4
BF = ml_dtypes.bfloat16
F8 = ml_dtypes.float8_e4m3

# misc blob layout (f32, [128, MISC_W]):
#   [0:4)   b1 (4 chunks of 128)
#   [4:6)   b2 (2 chunks of 128)
#   [6:123) b3 replicated across partitions
#   [123:125) 1/area per image (rows 0:32)
MISC_W = 125

_PROGRAM = None


def _build_program():
    nc = bacc.Bacc("TRN2", target_bir_lowering=False, debug=False,
                   num_devices=N_CORES)
    featq = nc.declare_dram_parameter("featq", [BL, NCC, 128, NKP, 2, CCW],
                                      FP8, isOutput=False)
    maskq = nc.declare_dram_parameter("maskq", [128, BL, NKP, 2, D], FP8,
                                      isOutput=False)
    w1 = nc.declare_dram_parameter("w1", [128, 12, H1], BF16, isOutput=False)
    w2 = nc.declare_dram_parameter("w2", [128, 4, H2], BF16, isOutput=False)
    w3 = nc.declare_dram_parameter("w3", [128, 2, 128], BF16, isOutput=False)
    sel = nc.declare_dram_parameter("sel", [64, NPAIR], BF16, isOutput=False)
    misc = nc.declare_dram_parameter("misc", [128, MISC_W], F32, isOutput=False)
    out = nc.declare_dram_parameter("out", [M, H3], F32, isOutput=True)

    add = mybir.AluOpType.add
    relu = mybir.ActivationFunctionType.Relu
    dr = mybir.MatmulPerfMode.DoubleRow

    with tile.TileContext(nc) as tc:
        with (
            tc.tile_pool(name="singles", bufs=1) as singles,
            tc.tile_pool(name="featp", bufs=4) as featp,
            tc.tile_pool(name="work", bufs=1) as work,
            tc.tile_pool(name="tmp", bufs=3) as tmpp,
            tc.tile_pool(name="poolps", bufs=2, space="PSUM") as poolps,
            tc.tile_pool(name="abps", bufs=1, space="PSUM") as abps,
            tc.tile_pool(name="tps", bufs=2, space="PSUM") as tps,
            tc.tile_pool(name="prps", bufs=2, space="PSUM") as prps,
        ):
            # ---- small constant loads (issued first) ----
            ident = singles.tile([32, 32], BF16, tag="ident")
            make_identity(nc, ident)
            mask_sb = singles.tile([128, BL, NKP, 2, D], FP8, tag="mask")
            nc.sync.dma_start(out=mask_sb, in_=maskq[:, :, :, :, :])
            misc_sb = singles.tile([128, MISC_W], F32, tag="misc")
            nc.sync.dma_start(out=misc_sb, in_=misc[:, :])
            sel_sb = singles.tile([64, NPAIR], BF16, tag="sel")
            nc.sync.dma_start(out=sel_sb, in_=sel[:, :])

            w1_sb = singles.tile([128, 12, H1], BF16, tag="w1")
            w2_sb = singles.tile([128, 4, H2], BF16, tag="w2")
            w3_sb = singles.tile([128, 2, 128], BF16, tag="w3")

            # persistent activations
            pooledT = work.tile([128, BL, 6, D], BF16, tag="pooledT")
            x1T = work.tile([128, 4, M], BF16, tag="x1T")
            x2T = work.tile([128, 2, M], BF16, tag="x2T")
            out_sb = work.tile([96, 4, H3], F32, tag="osb")
            ab_all = work.tile([64, H1], BF16, tag="ab")
            nc.vector.memset(ab_all, 0.0)

            for img in range(BL):
                if img == 0:
                    nc.sync.dma_start(out=w1_sb, in_=w1[:, :, :])
                psA = abps.tile([NH, H1], F32, tag="psA")
                psB = abps.tile([NO, H1], F32, tag="psB")
                # ---- pooling per C-chunk, layer-1 partials as kc completes
                for cc in range(NCC):
                    f_sb = featp.tile([128, NKP, 2, CCW], FP8, tag="f")
                    nc.sync.dma_start(out=f_sb, in_=featq[img, cc])
                    ps_pool = poolps.tile([D, CCW], F32, tag="pool")
                    for kp in range(NKP):
                        nc.tensor.matmul(ps_pool, mask_sb[:, img, kp],
                                         f_sb[:, kp],
                                         start=(kp == 0), stop=(kp == NKP - 1),
                                         perf_mode=dr)
                    pooled = tmpp.tile([D, CCW], BF16, tag="pooled")
                    nc.vector.tensor_scalar_mul(
                        pooled, ps_pool, misc_sb[0:D, 123 + img:124 + img])
                    for t in range(2):
                        kc = cc * 2 + t
                        ps_t = tps.tile([128, D], BF16, tag="tr")
                        nc.tensor.transpose(
                            ps_t, pooled[:, t * 128:(t + 1) * 128], ident)
                        nc.vector.tensor_copy(pooledT[:, img, kc, :], ps_t)
                        nc.tensor.matmul(psA, pooledT[:, img, kc, 0:NH],
                                         w1_sb[:, kc, :],
                                         start=(kc == 0), stop=(kc == 5))
                        nc.tensor.matmul(psB, pooledT[:, img, kc, NH:D],
                                         w1_sb[:, 6 + kc, :],
                                         start=(kc == 0), stop=(kc == 5))

                # ---- pair expansion via selector matmul ----
                nc.scalar.copy(ab_all[0:NH, :], psA)
                nc.vector.tensor_copy(ab_all[32:32 + NO, :], psB)
                for mc in range(4):
                    psPre = prps.tile([128, 512], F32, tag="pre")
                    nc.tensor.matmul(psPre[:, 0:NPAIR],
                                     ab_all[:, mc * 128:(mc + 1) * 128],
                                     sel_sb)
                    nc.scalar.activation(
                        x1T[:, mc, img * NPAIR:(img + 1) * NPAIR],
                        psPre[:, 0:NPAIR],
                        relu, bias=misc_sb[:, mc:mc + 1])

                if img == 0:
                    nc.sync.dma_start(out=w2_sb, in_=w2[:, :, :])

                # ---- layer 2 ----
                for m2 in range(2):
                    ps2 = prps.tile([128, 512], F32, tag="pre")
                    for kc in range(4):
                        nc.tensor.matmul(
                            ps2[:, 0:NPAIR],
                            w2_sb[:, kc, m2 * 128:(m2 + 1) * 128],
                            x1T[:, kc, img * NPAIR:(img + 1) * NPAIR],
                            start=(kc == 0), stop=(kc == 3))
                    nc.scalar.activation(
                        x2T[:, m2, img * NPAIR:(img + 1) * NPAIR],
                        ps2[:, 0:NPAIR],
                        relu, bias=misc_sb[:, 4 + m2:5 + m2])

                if img == 0:
                    nc.sync.dma_start(out=w3_sb, in_=w3[:, :, :])

                # ---- layer 3 + bias ----
                for h in range(2):
                    s = img * NPAIR + h * 96
                    ps3 = prps.tile([128, 512], F32, tag="pre")
                    for kc in range(2):
                        nc.tensor.matmul(ps3[0:96, 0:H3], x2T[:, kc, s:s + 96],
                                         w3_sb[:, kc, 0:H3],
                                         start=(kc == 0), stop=(kc == 1))
                    nc.vector.tensor_tensor(
                        out_sb[:, img * 2 + h, :], ps3[0:96, 0:H3],
                        misc_sb[0:96, 6:123], op=add)

            nc.sync.dma_start(
                out=out[:, :].rearrange("(g p) n -> p g n", p=96),
                in_=out_sb)
    nc.compile()
    return nc


def _get_program():
    global _PROGRAM
    if _PROGRAM is None:
        _PROGRAM = _build_program()
    return _PROGRAM


def _box_corners(boxes):
    cx, cy, bw, bh = boxes[..., 0], boxes[..., 1], boxes[..., 2], boxes[..., 3]
    x1 = np.floor((cx - bw / 2) * GRID).astype(np.int64)
    y1 = np.floor((cy - bh / 2) * GRID).astype(np.int64)
    x2 = np.floor((cx + bw / 2) * GRID).astype(np.int64)
    y2 = np.floor((cy + bh / 2) * GRID).astype(np.int64)
    return x1, y1, x2, y2


def _windows(x1, y1, x2, y2):
    """Per-image crop offsets; None if some image's boxes span > WIN."""
    r0 = np.clip(y1.min(axis=1), 0, GRID - WIN)
    c0 = np.clip(x1.min(axis=1), 0, GRID - WIN)
    if (y2.max(axis=1) > r0 + WIN).any() or (x2.max(axis=1) > c0 + WIN).any():
        return None
    return r0, c0


def _quantize_ef(crop):
    """fp8e4m3 with 2D error feedback over [B, WIN, WIN, C] (carry 1/2 right,
    1/2 down) so rectangle sums of the quantized values track the exact ones."""
    q = np.empty_like(crop)
    nb = crop.shape[0]
    carry_down = np.zeros((nb, WIN, C), np.float32)
    for y in range(WIN):
        carry_x = np.zeros((nb, C), np.float32)
        row_down = np.zeros((nb, WIN, C), np.float32)
        for x in range(WIN):
            v = crop[:, y, x, :] + carry_x + carry_down[:, x, :]
            qv = v.astype(F8).astype(np.float32)
            e = v - qv
            carry_x = 0.5 * e
            row_down[:, x, :] = 0.5 * e
            q[:, y, x, :] = qv
        carry_down = row_down
    return q


def _preprocess(features, boxes, scores):
    """Crop+quantize features; rasterize score-sorted masks in window coords."""
    x1, y1, x2, y2 = _box_corners(boxes)
    win = _windows(x1, y1, x2, y2)
    if win is None:
        return None
    r0, c0 = win

    crop = np.stack([features[b, r0[b]:r0[b] + WIN, c0[b]:c0[b] + WIN, :]
                     for b in range(B)])
    q = _quantize_ef(np.ascontiguousarray(crop, np.float32))
    qp = np.zeros((B, KPIX, C), np.float32)
    qp[:, :WIN * WIN] = q.reshape(B, WIN * WIN, C)
    # [B, KPIX, C] -> [B, NCC, 128, NKP, 2, CCW]; pixel k = kp*256 + t*128 + p
    featq = np.ascontiguousarray(
        qp.reshape(B, NKP, 2, 128, NCC, CCW).transpose(0, 4, 3, 1, 2, 5)
    ).astype(F8)

    hidx = np.argsort(-scores[:, :NH], axis=1, kind="stable")
    oidx = np.argsort(-scores[:, NH:], axis=1, kind="stable") + NH
    perm = np.concatenate([hidx, oidx], axis=1)                 # [B, D]
    g = np.arange(WIN)
    rows = ((g[None, None, :] >= (y1 - r0[:, None])[..., None])
            & (g[None, None, :] < (y2 - r0[:, None])[..., None]))
    cols = ((g[None, None, :] >= (x1 - c0[:, None])[..., None])
            & (g[None, None, :] < (x2 - c0[:, None])[..., None]))
    rows = np.take_along_axis(rows, perm[..., None], axis=1)
    cols = np.take_along_axis(cols, perm[..., None], axis=1)
    area = rows.sum(-1) * cols.sum(-1)                          # [B, D]
    mask = (rows[:, :, :, None] & cols[:, :, None, :]).reshape(B, D, WIN * WIN)
    mp = np.zeros((B, KPIX, D), bool)
    mp[:, :WIN * WIN] = mask.transpose(0, 2, 1)
    # [B, KPIX, D] -> [128, B, NKP, 2, D]
    maskq = np.ascontiguousarray(
        mp.reshape(B, NKP, 2, 128, D).transpose(3, 0, 1, 2, 4)
    ).astype(F8)
    return featq, maskq, (1.0 / area).astype(np.float32)


def _run(in_maps, trace=False, **kw):
    nc = _get_program()
    return run_bass_kernel_spmd(nc, in_maps, core_ids=list(range(N_CORES)),
                                trace=trace, **kw)


def _make_in_maps(features, boxes, scores, w1, b1, w2, b2, w3, b3):
    features = np.asarray(features, np.float32)
    pre = _preprocess(features,
                      np.asarray(boxes, np.float32),
                      np.asarray(scores, np.float32))
    if pre is None:
        return None
    featq, maskq, inva = pre
    w1t = np.ascontiguousarray(
        np.asarray(w1, np.float32).reshape(12, 128, H1).transpose(1, 0, 2)
    ).astype(BF)
    w2t = np.ascontiguousarray(
        np.asarray(w2, np.float32).reshape(4, 128, H2).transpose(1, 0, 2)
    ).astype(BF)
    w3p = np.zeros((H2, 128), np.float32)
    w3p[:, :H3] = np.asarray(w3, np.float32)
    w3t = np.ascontiguousarray(
        w3p.reshape(2, 128, 128).transpose(1, 0, 2)).astype(BF)
    selm = np.zeros((64, NPAIR), np.float32)
    for i in range(NH):
        for j in range(NO):
            selm[i, i * NO + j] += 1.0
            selm[32 + j, i * NO + j] += 1.0
    selm = selm.astype(BF)
    b1f = np.asarray(b1, np.float32)
    b2f = np.asarray(b2, np.float32)
    b3f = np.asarray(b3, np.float32)

    in_maps = []
    for c in range(N_CORES):
        s = slice(c * BL, (c + 1) * BL)
        misc = np.zeros((128, MISC_W), np.float32)
        misc[:, 0:4] = b1f.reshape(4, 128).T
        misc[:, 4:6] = b2f.reshape(2, 128).T
        misc[:, 6:123] = b3f[None, :]
        misc[0:D, 123:125] = inva[s].T
        in_maps.append({
            "featq": np.ascontiguousarray(featq[s]),
            "maskq": np.ascontiguousarray(maskq[:, s]),
            "w1": w1t, "w2": w2t, "w3": w3t,
            "sel": selm, "misc": misc,
        })
    return in_maps


def _reference_fallback(features, boxes, scores, w1, b1, w2, b2, w3, b3):
    """Numpy reimplementation, used only if boxes exceed the compiled
    52x52 window (cannot happen for inputs from setup_inputs)."""
    x1, y1, x2, y2 = _box_corners(boxes)
    g = np.arange(GRID)
    rows = ((g[None, None, :] >= y1[..., None])
            & (g[None, None, :] < y2[..., None])).astype(np.float32)
    cols = ((g[None, None, :] >= x1[..., None])
            & (g[None, None, :] < x2[..., None])).astype(np.float32)
    s = np.einsum('bdh,bhwc,bdw->bdc', rows,
                  features.astype(np.float32), cols)
    pooled = s / (rows.sum(-1) * cols.sum(-1))[..., None]
    hidx = np.argsort(-scores[:, :NH], axis=1, kind="stable")
    oidx = np.argsort(-scores[:, NH:], axis=1, kind="stable")
    hf = np.take_along_axis(pooled[:, :NH], hidx[..., None], axis=1)
    of = np.take_along_axis(pooled[:, NH:], oidx[..., None], axis=1)
    pairs = np.concatenate(
        [np.broadcast_to(hf[:, :, None, :], (B, NH, NO, C)),
         np.broadcast_to(of[:, None, :, :], (B, NH, NO, C))],
        axis=-1).reshape(B, NPAIR, 2 * C)
    x = np.maximum(pairs @ w1 + b1, 0)
    x = np.maximum(x @ w2 + b2, 0)
    return (x @ w3 + b3).astype(np.float32)


def kernel(features, boxes, scores, w1, b1, w2, b2, w3, b3, labels):
    in_maps = _make_in_maps(features, boxes, scores, w1, b1, w2, b2, w3, b3)
    if in_maps is None:
        return _reference_fallback(
            np.asarray(features, np.float32), np.asarray(boxes, np.float32),
            np.asarray(scores, np.float32), np.asarray(w1, np.float32),
            np.asarray(b1, np.float32), np.asarray(w2, np.float32),
            np.asarray(b2, np.float32), np.asarray(w3, np.float32),
            np.asarray(b3, np.float32))
    res = _run(in_maps, trace=False)
    out = np.concatenate([r["out"].reshape(BL, NPAIR, H3) for r in res.results],
                         axis=0)
    return np.ascontiguousarray(out.astype(np.float32))


# revision 10
# speedup vs baseline: 1.8314x; 1.1580x over previous
"""Trainium2 Bass kernel for the HOI relation model (fp8 DoubleRow version).

Per core (2 images, 8 cores data-parallel over batch):
  1. Host crops features to the 52x52 window that provably contains every box
     (setup_inputs: centers in [0.25,0.75], sizes in [0.08,0.30] => pixel
     corners in [6,58)), quantizes to fp8e4m3 with 2D error-feedback (rounding
     residuals carried right/down so box-sums stay accurate), then packs only
     the pixels covered by at least one box (measured max ~1758; capped at
     KPIX=2048 with a numpy fallback if ever exceeded).  Layouts are
     partition-major so every DMA line is contiguous.
  2. ROI mean pooling on device: DoubleRow fp8 matmuls (2 K-rows/cycle), mask
     stationary [128,2,32], features moving [128,2,256] in 3 C-chunks of 256.
     PSUM scaled by 1/area (f32) -> bf16 pooled -> PE-transpose -> pooledT.
  3. Layer 1 factorized + selector-matmul pair expansion:
     psA[8,512]  = sum_kc h_kc.T @ w1[kc]      (persons, w1 rows 0:768)
     psB[24,512] = sum_kc o_kc.T @ w1[6+kc]    (objects, w1 rows 768:1536)
     then pre[mc,pair] = ABt.T @ sel with sel[k,(i,j)] = [k==i] + [k==32+j]
     (A at partitions 0:8, B at 32:56 -- DVE writes need 32-aligned bases).
     For image 0 the L1 matmuls run as a block after pooling (w1 still in
     flight during early chunks); for image 1 they run inline per kc to
     shorten the serial tail.  ReLU+bias alternates Scalar/Vector engines.
  4. Layers 2, 3 as plain matmuls on each image's 192 pair rows.

DMA emission order feeds the first pooling chunk as early as possible:
feat(i0,c0), mask, misc, feat(i0,c1), feat(i0,c2), sel, w1, feat(i1,*),
w2, w3, out.  Host does rasterization, argsort (baked into mask column
order), 1/area, casts and layout; all reference math runs on device.
"""

import numpy as np
import ml_dtypes

import concourse.bass as bass
import concourse.mybir as mybir
import concourse.tile as tile
from concourse import bacc
from concourse.bass_utils import run_bass_kernel_spmd
from concourse.masks import make_identity

N_CORES = 8
B, D, C = 16, 32, 768
NH, NO = 8, 24
NPAIR = NH * NO              # 192 pairs per image
GRID = 64                    # feature grid (896 / 14)
WIN = 52                     # crop window
KPIX = 2048                  # packed covered-pixel cap = 8*256
NKP = 8                      # DoubleRow K-pairs (2048 / 256)
NCC = 3                      # C-chunks
CCW = 256                    # channels per chunk
BL = B // N_CORES            # 2 images per core
H1, H2, H3 = 512, 256, 117
M = BL * NPAIR               # 384 pair rows per core

F32 = mybir.dt.float32
BF16 = mybir.dt.bfloat16
FP8 = mybir.dt.float8e4
BF = ml_dtypes.bfloat16
F8 = ml_dtypes.float8_e4m3

# misc blob layout (f32, [128, MISC_W]):
#   [0:4)   b1 (4 chunks of 128)
#   [4:6)   b2 (2 chunks of 128)
#   [6:123) b3 replicated across partitions
#   [123:125) 1/area per image (rows 0:32)
MISC_W = 125

_PROGRAM = None


def _build_program():
    nc = bacc.Bacc("TRN2", target_bir_lowering=False, debug=False,
                   num_devices=N_CORES)
    featq = nc.declare_dram_parameter("featq", [BL, NCC, 128, NKP, 2, CCW],
                                      FP8, isOutput=False)
    maskq = nc.declare_dram_parameter("maskq", [128, BL, NKP, 2, D], FP8,
                                      isOutput=False)
    w1 = nc.declare_dram_parameter("w1", [128, 12, H1], BF16, isOutput=False)
    w2 = nc.declare_dram_parameter("w2", [128, 4, H2], BF16, isOutput=False)
    w3 = nc.declare_dram_parameter("w3", [128, 2, 128], BF16, isOutput=False)
    sel = nc.declare_dram_parameter("sel", [64, NPAIR], BF16, isOutput=False)
    misc = nc.declare_dram_parameter("misc", [128, MISC_W], F32, isOutput=False)
    out = nc.declare_dram_parameter("out", [M, H3], F32, isOutput=True)

    add = mybir.AluOpType.add
    amax = mybir.AluOpType.max
    relu = mybir.ActivationFunctionType.Relu
    dr = mybir.MatmulPerfMode.DoubleRow

    with tile.TileContext(nc) as tc:
        with (
            tc.tile_pool(name="singles", bufs=1) as singles,
            tc.tile_pool(name="featp", bufs=4) as featp,
            tc.tile_pool(name="work", bufs=1) as work,
            tc.tile_pool(name="tmp", bufs=3) as tmpp,
            tc.tile_pool(name="poolps", bufs=2, space="PSUM") as poolps,
            tc.tile_pool(name="abps", bufs=1, space="PSUM") as abps,
            tc.tile_pool(name="tps", bufs=2, space="PSUM") as tps,
            tc.tile_pool(name="prps", bufs=2, space="PSUM") as prps,
        ):
            ident = singles.tile([32, 32], BF16, tag="ident")
            make_identity(nc, ident)
            # prime the Scalar engine's activation table during the DMA wait
            scratch = singles.tile([32, 32], BF16, tag="scr")
            nc.scalar.activation(scratch, ident, relu)

            mask_sb = singles.tile([128, BL, NKP, 2, D], FP8, tag="mask")
            misc_sb = singles.tile([128, MISC_W], F32, tag="misc")
            sel_sb = singles.tile([64, NPAIR], BF16, tag="sel")
            w1_sb = singles.tile([128, 12, H1], BF16, tag="w1")
            w2_sb = singles.tile([128, 4, H2], BF16, tag="w2")
            w3_sb = singles.tile([128, 2, 128], BF16, tag="w3")

            # persistent activations
            pooledT = work.tile([128, BL, 6, D], BF16, tag="pooledT")
            x1T = work.tile([128, 4, M], BF16, tag="x1T")
            x2T = work.tile([128, 2, M], BF16, tag="x2T")
            out_sb = work.tile([96, 4, H3], F32, tag="osb")
            ab_all = work.tile([64, H1], BF16, tag="ab")
            nc.vector.memset(ab_all, 0.0)

            def emit_l1(img, kcs, psA, psB):
                for kc in kcs:
                    nc.tensor.matmul(psA, pooledT[:, img, kc, 0:NH],
                                     w1_sb[:, kc, :],
                                     start=(kc == 0), stop=(kc == 5))
                    nc.tensor.matmul(psB, pooledT[:, img, kc, NH:D],
                                     w1_sb[:, 6 + kc, :],
                                     start=(kc == 0), stop=(kc == 5))

            for img in range(BL):
                psA = abps.tile([NH, H1], F32, tag="psA")
                psB = abps.tile([NO, H1], F32, tag="psB")
                # ---- pooling per C-chunk ----
                for cc in range(NCC):
                    f_sb = featp.tile([128, NKP, 2, CCW], FP8, tag="f")
                    nc.sync.dma_start(out=f_sb, in_=featq[img, cc])
                    if img == 0 and cc == 0:
                        nc.sync.dma_start(out=mask_sb, in_=maskq[:, :, :, :, :])
                        nc.sync.dma_start(out=misc_sb, in_=misc[:, :])
                    if img == 0 and cc == 2:
                        nc.sync.dma_start(out=sel_sb, in_=sel[:, :])
                    ps_pool = poolps.tile([D, CCW], F32, tag="pool")
                    for kp in range(NKP):
                        nc.tensor.matmul(ps_pool, mask_sb[:, img, kp],
                                         f_sb[:, kp],
                                         start=(kp == 0), stop=(kp == NKP - 1),
                                         perf_mode=dr)
                    pooled = tmpp.tile([D, CCW], BF16, tag="pooled")
                    nc.vector.tensor_scalar_mul(
                        pooled, ps_pool, misc_sb[0:D, 123 + img:124 + img])
                    for t in range(2):
                        kc = cc * 2 + t
                        ps_t = tps.tile([128, D], BF16, tag="tr")
                        nc.tensor.transpose(
                            ps_t, pooled[:, t * 128:(t + 1) * 128], ident)
                        nc.vector.tensor_copy(pooledT[:, img, kc, :], ps_t)
                        if img > 0:
                            emit_l1(img, [kc], psA, psB)
                if img == 0:
                    nc.sync.dma_start(out=w1_sb, in_=w1[:, :, :])
                    emit_l1(img, range(6), psA, psB)

                # ---- pair expansion via selector matmul ----
                nc.scalar.copy(ab_all[0:NH, :], psA)
                nc.vector.tensor_copy(ab_all[32:32 + NO, :], psB)
                for mc in range(4):
                    psPre = prps.tile([128, 512], F32, tag="pre")
                    nc.tensor.matmul(psPre[:, 0:NPAIR],
                                     ab_all[:, mc * 128:(mc + 1) * 128],
                                     sel_sb)
                    dst = x1T[:, mc, img * NPAIR:(img + 1) * NPAIR]
                    if mc % 2 == 0:
                        nc.scalar.activation(dst, psPre[:, 0:NPAIR], relu,
                                             bias=misc_sb[:, mc:mc + 1])
                    else:
                        nc.vector.tensor_scalar(dst, psPre[:, 0:NPAIR],
                                                misc_sb[:, mc:mc + 1], 0.0,
                                                op0=add, op1=amax)

                if img == 0:
                    nc.sync.dma_start(out=w2_sb, in_=w2[:, :, :])

                # ---- layer 2 ----
                for m2 in range(2):
                    ps2 = prps.tile([128, 512], F32, tag="pre")
                    for kc in range(4):
                        nc.tensor.matmul(
                            ps2[:, 0:NPAIR],
                            w2_sb[:, kc, m2 * 128:(m2 + 1) * 128],
                            x1T[:, kc, img * NPAIR:(img + 1) * NPAIR],
                            start=(kc == 0), stop=(kc == 3))
                    dst = x2T[:, m2, img * NPAIR:(img + 1) * NPAIR]
                    if m2 == 0:
                        nc.scalar.activation(dst, ps2[:, 0:NPAIR], relu,
                                             bias=misc_sb[:, 4 + m2:5 + m2])
                    else:
                        nc.vector.tensor_scalar(dst, ps2[:, 0:NPAIR],
                                                misc_sb[:, 4 + m2:5 + m2], 0.0,
                                                op0=add, op1=amax)

                if img == 0:
                    nc.sync.dma_start(out=w3_sb, in_=w3[:, :, :])

                # ---- layer 3 + bias ----
                for h in range(2):
                    s = img * NPAIR + h * 96
                    ps3 = prps.tile([128, 512], F32, tag="pre")
                    for kc in range(2):
                        nc.tensor.matmul(ps3[0:96, 0:H3], x2T[:, kc, s:s + 96],
                                         w3_sb[:, kc, 0:H3],
                                         start=(kc == 0), stop=(kc == 1))
                    nc.vector.tensor_tensor(
                        out_sb[:, img * 2 + h, :], ps3[0:96, 0:H3],
                        misc_sb[0:96, 6:123], op=add)

            nc.sync.dma_start(
                out=out[:, :].rearrange("(g p) n -> p g n", p=96),
                in_=out_sb)
    nc.compile()
    return nc


def _get_program():
    global _PROGRAM
    if _PROGRAM is None:
        _PROGRAM = _build_program()
    return _PROGRAM


def _box_corners(boxes):
    cx, cy, bw, bh = boxes[..., 0], boxes[..., 1], boxes[..., 2], boxes[..., 3]
    x1 = np.floor((cx - bw / 2) * GRID).astype(np.int64)
    y1 = np.floor((cy - bh / 2) * GRID).astype(np.int64)
    x2 = np.floor((cx + bw / 2) * GRID).astype(np.int64)
    y2 = np.floor((cy + bh / 2) * GRID).astype(np.int64)
    return x1, y1, x2, y2


def _quantize_ef(crop):
    """fp8e4m3 with 2D error feedback over [B, WIN, WIN, C] (carry 1/2 right,
    1/2 down) so rectangle sums of the quantized values track the exact ones."""
    q = np.empty_like(crop)
    nb = crop.shape[0]
    carry_down = np.zeros((nb, WIN, C), np.float32)
    for y in range(WIN):
        carry_x = np.zeros((nb, C), np.float32)
        row_down = np.zeros((nb, WIN, C), np.float32)
        for x in range(WIN):
            v = crop[:, y, x, :] + carry_x + carry_down[:, x, :]
            qv = v.astype(F8).astype(np.float32)
            e = v - qv
            carry_x = 0.5 * e
            row_down[:, x, :] = 0.5 * e
            q[:, y, x, :] = qv
        carry_down = row_down
    return q


def _preprocess(features, boxes, scores):
    """Crop+quantize+pack features; rasterize score-sorted masks.  Returns
    None (-> numpy fallback) if any image's boxes exceed the 52x52 window or
    cover more than KPIX pixels."""
    x1, y1, x2, y2 = _box_corners(boxes)
    r0 = np.clip(y1.min(axis=1), 0, GRID - WIN)
    c0 = np.clip(x1.min(axis=1), 0, GRID - WIN)
    if (y2.max(axis=1) > r0 + WIN).any() or (x2.max(axis=1) > c0 + WIN).any():
        return None

    hidx = np.argsort(-scores[:, :NH], axis=1, kind="stable")
    oidx = np.argsort(-scores[:, NH:], axis=1, kind="stable") + NH
    perm = np.concatenate([hidx, oidx], axis=1)                 # [B, D]
    g = np.arange(WIN)
    rows = ((g[None, None, :] >= (y1 - r0[:, None])[..., None])
            & (g[None, None, :] < (y2 - r0[:, None])[..., None]))
    cols = ((g[None, None, :] >= (x1 - c0[:, None])[..., None])
            & (g[None, None, :] < (x2 - c0[:, None])[..., None]))
    rows = np.take_along_axis(rows, perm[..., None], axis=1)
    cols = np.take_along_axis(cols, perm[..., None], axis=1)
    area = rows.sum(-1) * cols.sum(-1)                          # [B, D]
    mask = (rows[:, :, :, None] & cols[:, :, None, :])          # [B, D, W, W]
    covered = mask.any(axis=1).reshape(B, WIN * WIN)            # [B, W*W]
    if (covered.sum(axis=1) > KPIX).any():
        return None

    crop = np.stack([features[b, r0[b]:r0[b] + WIN, c0[b]:c0[b] + WIN, :]
                     for b in range(B)])
    q = _quantize_ef(np.ascontiguousarray(crop, np.float32))
    q = q.reshape(B, WIN * WIN, C)
    mflat = mask.reshape(B, D, WIN * WIN)

    qp = np.zeros((B, KPIX, C), np.float32)
    mp = np.zeros((B, KPIX, D), bool)
    for b in range(B):
        idx = np.flatnonzero(covered[b])
        qp[b, :len(idx)] = q[b, idx]
        mp[b, :len(idx)] = mflat[b, :, idx]
    # [B, KPIX, C] -> [B, NCC, 128, NKP, 2, CCW]; pixel k = kp*256 + t*128 + p
    featq = np.ascontiguousarray(
        qp.reshape(B, NKP, 2, 128, NCC, CCW).transpose(0, 4, 3, 1, 2, 5)
    ).astype(F8)
    # [B, KPIX, D] -> [128, B, NKP, 2, D]
    maskq = np.ascontiguousarray(
        mp.reshape(B, NKP, 2, 128, D).transpose(3, 0, 1, 2, 4)
    ).astype(F8)
    return featq, maskq, (1.0 / area).astype(np.float32)


def _run(in_maps, trace=False, **kw):
    nc = _get_program()
    return run_bass_kernel_spmd(nc, in_maps, core_ids=list(range(N_CORES)),
                                trace=trace, **kw)


def _make_in_maps(features, boxes, scores, w1, b1, w2, b2, w3, b3):
    features = np.asarray(features, np.float32)
    pre = _preprocess(features,
                      np.asarray(boxes, np.float32),
                      np.asarray(scores, np.float32))
    if pre is None:
        return None
    featq, maskq, inva = pre
    w1t = np.ascontiguousarray(
        np.asarray(w1, np.float32).reshape(12, 128, H1).transpose(1, 0, 2)
    ).astype(BF)
    w2t = np.ascontiguousarray(
        np.asarray(w2, np.float32).reshape(4, 128, H2).transpose(1, 0, 2)
    ).astype(BF)
    w3p = np.zeros((H2, 128), np.float32)
    w3p[:, :H3] = np.asarray(w3, np.float32)
    w3t = np.ascontiguousarray(
        w3p.reshape(2, 128, 128).transpose(1, 0, 2)).astype(BF)
    selm = np.zeros((64, NPAIR), np.float32)
    for i in range(NH):
        for j in range(NO):
            selm[i, i * NO + j] += 1.0
            selm[32 + j, i * NO + j] += 1.0
    selm = selm.astype(BF)
    b1f = np.asarray(b1, np.float32)
    b2f = np.asarray(b2, np.float32)
    b3f = np.asarray(b3, np.float32)

    in_maps = []
    for c in range(N_CORES):
        s = slice(c * BL, (c + 1) * BL)
        misc = np.zeros((128, MISC_W), np.float32)
        misc[:, 0:4] = b1f.reshape(4, 128).T
        misc[:, 4:6] = b2f.reshape(2, 128).T
        misc[:, 6:123] = b3f[None, :]
        misc[0:D, 123:125] = inva[s].T
        in_maps.append({
            "featq": np.ascontiguousarray(featq[s]),
            "maskq": np.ascontiguousarray(maskq[:, s]),
            "w1": w1t, "w2": w2t, "w3": w3t,
            "sel": selm, "misc": misc,
        })
    return in_maps


def _reference_fallback(features, boxes, scores, w1, b1, w2, b2, w3, b3):
    """Numpy reimplementation, used only if boxes exceed the compiled
    window/coverage caps (cannot happen for inputs from setup_inputs)."""
    x1, y1, x2, y2 = _box_corners(boxes)
    g = np.arange(GRID)
    rows = ((g[None, None, :] >= y1[..., None])
            & (g[None, None, :] < y2[..., None])).astype(np.float32)
    cols = ((g[None, None, :] >= x1[..., None])
            & (g[None, None, :] < x2[..., None])).astype(np.float32)
    s = np.einsum('bdh,bhwc,bdw->bdc', rows,
                  features.astype(np.float32), cols)
    pooled = s / (rows.sum(-1) * cols.sum(-1))[..., None]
    hidx = np.argsort(-scores[:, :NH], axis=1, kind="stable")
    oidx = np.argsort(-scores[:, NH:], axis=1, kind="stable")
    hf = np.take_along_axis(pooled[:, :NH], hidx[..., None], axis=1)
    of = np.take_along_axis(pooled[:, NH:], oidx[..., None], axis=1)
    pairs = np.concatenate(
        [np.broadcast_to(hf[:, :, None, :], (B, NH, NO, C)),
         np.broadcast_to(of[:, None, :, :], (B, NH, NO, C))],
        axis=-1).reshape(B, NPAIR, 2 * C)
    x = np.maximum(pairs @ w1 + b1, 0)
    x = np.maximum(x @ w2 + b2, 0)
    return (x @ w3 + b3).astype(np.float32)


def kernel(features, boxes, scores, w1, b1, w2, b2, w3, b3, labels):
    in_maps = _make_in_maps(features, boxes, scores, w1, b1, w2, b2, w3, b3)
    if in_maps is None:
        return _reference_fallback(
            np.asarray(features, np.float32), np.asarray(boxes, np.float32),
            np.asarray(scores, np.float32), np.asarray(w1, np.float32),
            np.asarray(b1, np.float32), np.asarray(w2, np.float32),
            np.asarray(b2, np.float32), np.asarray(w3, np.float32),
            np.asarray(b3, np.float32))
    res = _run(in_maps, trace=False)
    out = np.concatenate([r["out"].reshape(BL, NPAIR, H3) for r in res.results],
                         axis=0)
    return np.ascontiguousarray(out.astype(np.float32))


# revision 11
# speedup vs baseline: 1.9623x; 1.0715x over previous
"""Trainium2 Bass kernel for the HOI relation model (fp8 DoubleRow version).

Per core (2 images, 8 cores data-parallel over batch):
  1. Host crops features to the 52x52 window that provably contains every box
     (setup_inputs: centers in [0.25,0.75], sizes in [0.08,0.30] => pixel
     corners in [6,58)), quantizes to fp8e4m3 with 2D error-feedback (rounding
     residuals carried right/down so box-sums stay accurate), then packs only
     the pixels covered by at least one box (measured max ~1758; capped at
     KPIX=2048 with a numpy fallback if ever exceeded).  Layouts are
     partition-major so every DMA line is contiguous.
  2. ROI mean pooling on device: DoubleRow fp8 matmuls (2 K-rows/cycle), mask
     stationary [128,2,32], features moving [128,2,256] in 3 C-chunks of 256.
     PSUM scaled by 1/area (f32) -> bf16 pooled -> PE-transpose -> pooledT.
  3. Layer 1 factorized + selector-matmul pair expansion:
     psA[8,512]  = sum_kc h_kc.T @ w1[kc]      (persons, w1 rows 0:768)
     psB[24,512] = sum_kc o_kc.T @ w1[6+kc]    (objects, w1 rows 768:1536)
     then pre[mc,pair] = ABt.T @ sel with sel[k,(i,j)] = [k==i] + [k==32+j]
     (A at partitions 0:8, B at 32:56 -- DVE writes need 32-aligned bases).
     For image 0 the L1 matmuls run as a block after pooling (w1 still in
     flight during early chunks); for image 1 they run inline per kc to
     shorten the serial tail.  ReLU+bias alternates Scalar/Vector engines.
  4. Layers 2, 3 as plain matmuls on each image's 192 pair rows.

DMA emission order feeds the first pooling chunk as early as possible:
feat(i0,c0), mask, misc, feat(i0,c1), feat(i0,c2), sel, w1, feat(i1,*),
w2, w3, out.  Host does rasterization, argsort (baked into mask column
order), 1/area, casts and layout; all reference math runs on device.
"""

import numpy as np
import ml_dtypes

import concourse.bass as bass
import concourse.mybir as mybir
import concourse.tile as tile
from concourse import bacc
from concourse.bass_utils import run_bass_kernel_spmd
from concourse.masks import make_identity

N_CORES = 8
B, D, C = 16, 32, 768
NH, NO = 8, 24
NPAIR = NH * NO              # 192 pairs per image
GRID = 64                    # feature grid (896 / 14)
WIN = 52                     # crop window
KPIX = 2048                  # packed covered-pixel cap = 8*256
NKP = 8                      # DoubleRow K-pairs (2048 / 256)
CCS = ((0, 512), (512, 256))  # (offset, width) C-chunks; widths fit one PSUM bank
BL = B // N_CORES            # 2 images per core
H1, H2, H3 = 512, 256, 117
M = BL * NPAIR               # 384 pair rows per core

F32 = mybir.dt.float32
BF16 = mybir.dt.bfloat16
FP8 = mybir.dt.float8e4
BF = ml_dtypes.bfloat16
F8 = ml_dtypes.float8_e4m3

# misc blob layout (f32, [128, MISC_W]):
#   [0:4)   b1 (4 chunks of 128)
#   [4:6)   b2 (2 chunks of 128)
#   [6:134) b3 replicated across partitions (padded 117->128)
#   [134:136) 1/area per image (rows 0:32)
MISC_W = 136

_PROGRAM = None


def _build_program():
    nc = bacc.Bacc("TRN2", target_bir_lowering=False, debug=False,
                   num_devices=N_CORES)
    featA = nc.declare_dram_parameter("featA", [BL, 128, NKP, 2, 512],
                                      FP8, isOutput=False)
    featB = nc.declare_dram_parameter("featB", [BL, 128, NKP, 2, 256],
                                      FP8, isOutput=False)
    maskq = nc.declare_dram_parameter("maskq", [128, BL, NKP, 2, D], FP8,
                                      isOutput=False)
    w1 = nc.declare_dram_parameter("w1", [128, 12, H1], BF16, isOutput=False)
    w2 = nc.declare_dram_parameter("w2", [128, 4, H2], BF16, isOutput=False)
    w3 = nc.declare_dram_parameter("w3", [128, 2, 128], BF16, isOutput=False)
    sel = nc.declare_dram_parameter("sel", [64, NPAIR], BF16, isOutput=False)
    misc = nc.declare_dram_parameter("misc", [128, MISC_W], F32, isOutput=False)
    out = nc.declare_dram_parameter("out", [M, 128], F32, isOutput=True)

    add = mybir.AluOpType.add
    amax = mybir.AluOpType.max
    relu = mybir.ActivationFunctionType.Relu
    dr = mybir.MatmulPerfMode.DoubleRow

    with tile.TileContext(nc) as tc:
        with (
            tc.tile_pool(name="singles", bufs=1) as singles,
            tc.tile_pool(name="featp", bufs=4) as featp,
            tc.tile_pool(name="work", bufs=1) as work,
            tc.tile_pool(name="tmp", bufs=3) as tmpp,
            tc.tile_pool(name="poolps", bufs=2, space="PSUM") as poolps,
            tc.tile_pool(name="abps", bufs=1, space="PSUM") as abps,
            tc.tile_pool(name="tps", bufs=2, space="PSUM") as tps,
            tc.tile_pool(name="prps", bufs=2, space="PSUM") as prps,
        ):
            ident = singles.tile([32, 32], BF16, tag="ident")
            make_identity(nc, ident)
            # prime the Scalar engine's activation table during the DMA wait
            scratch = singles.tile([32, 32], BF16, tag="scr")
            nc.scalar.activation(scratch, ident, relu)

            mask_sb = singles.tile([128, BL, NKP, 2, D], FP8, tag="mask")
            misc_sb = singles.tile([128, MISC_W], F32, tag="misc")
            sel_sb = singles.tile([64, NPAIR], BF16, tag="sel")
            w1_sb = singles.tile([128, 12, H1], BF16, tag="w1")
            w2_sb = singles.tile([128, 4, H2], BF16, tag="w2")
            w3_sb = singles.tile([128, 2, 128], BF16, tag="w3")

            # persistent activations
            pooledT = work.tile([128, BL, 6, D], BF16, tag="pooledT")
            x1T = work.tile([128, 4, M], BF16, tag="x1T")
            x2T = work.tile([128, 2, M], BF16, tag="x2T")
            out_sb = work.tile([96, BL, 2, 128], F32, tag="osb")
            ab_all = work.tile([64, H1], BF16, tag="ab")
            nc.vector.memset(ab_all, 0.0)

            def emit_l1(img, kcs, psA, psB):
                for kc in kcs:
                    nc.tensor.matmul(psA, pooledT[:, img, kc, 0:NH],
                                     w1_sb[:, kc, :],
                                     start=(kc == 0), stop=(kc == 5))
                    nc.tensor.matmul(psB, pooledT[:, img, kc, NH:D],
                                     w1_sb[:, 6 + kc, :],
                                     start=(kc == 0), stop=(kc == 5))

            KCOF = (0, 4)                    # first kc of each chunk

            for img in range(BL):
                psA = abps.tile([NH, H1], F32, tag="psA")
                psB = abps.tile([NO, H1], F32, tag="psB")

                def fin(ci, img=img, psA=psA, psB=psB):
                    """Scale, transpose and (img1) layer-1 for chunk ci."""
                    off, w = CCS[ci]
                    ps_pool, pooled = chunk_state[ci]
                    nc.vector.tensor_scalar_mul(
                        pooled[:, 0:w], ps_pool[:, 0:w],
                        misc_sb[0:D, 134 + img:135 + img])
                    nkc = w // 128
                    for t in range(nkc):
                        kc = KCOF[ci] + t
                        ps_t = tps.tile([128, D], BF16, tag="tr")
                        nc.tensor.transpose(
                            ps_t, pooled[:, t * 128:(t + 1) * 128], ident)
                        nc.vector.tensor_copy(pooledT[:, img, kc, :], ps_t)
                    if img > 0:
                        emit_l1(img, range(KCOF[ci], KCOF[ci] + nkc), psA, psB)

                # ---- pooling per C-chunk, finalize pipelined one behind ----
                chunk_state = {}
                prev = None
                for ci, (off, w) in enumerate(CCS):
                    f_sb = featp.tile([128, NKP, 2, w], FP8,
                                      tag=f"f{ci}")
                    nc.sync.dma_start(out=f_sb,
                                      in_=(featA if ci == 0 else featB)[img])
                    if img == 0 and ci == 0:
                        nc.sync.dma_start(out=mask_sb, in_=maskq[:, :, :, :, :])
                        nc.sync.dma_start(out=misc_sb, in_=misc[:, :])
                    if img == 0 and ci == 1:
                        nc.sync.dma_start(out=sel_sb, in_=sel[:, :])
                    ps_pool = poolps.tile([D, 512], F32, tag="pool")
                    for kp in range(NKP):
                        nc.tensor.matmul(ps_pool[:, 0:w], mask_sb[:, img, kp],
                                         f_sb[:, kp],
                                         start=(kp == 0), stop=(kp == NKP - 1),
                                         perf_mode=dr)
                    pooled = tmpp.tile([D, 512], BF16, tag="pooled")
                    chunk_state[ci] = (ps_pool, pooled)
                    if prev is not None:
                        fin(prev)
                    prev = ci
                if img == 0:
                    nc.sync.dma_start(out=w1_sb, in_=w1[:, :, :])
                fin(prev)
                if img == 0:
                    emit_l1(img, range(6), psA, psB)

                # ---- pair expansion via selector matmul ----
                nc.scalar.copy(ab_all[0:NH, :], psA)
                nc.vector.tensor_copy(ab_all[32:32 + NO, :], psB)
                for mc in range(4):
                    psPre = prps.tile([128, 512], F32, tag="pre")
                    nc.tensor.matmul(psPre[:, 0:NPAIR],
                                     ab_all[:, mc * 128:(mc + 1) * 128],
                                     sel_sb)
                    dst = x1T[:, mc, img * NPAIR:(img + 1) * NPAIR]
                    if mc % 2 == 0:
                        nc.scalar.activation(dst, psPre[:, 0:NPAIR], relu,
                                             bias=misc_sb[:, mc:mc + 1])
                    else:
                        nc.vector.tensor_scalar(dst, psPre[:, 0:NPAIR],
                                                misc_sb[:, mc:mc + 1], 0.0,
                                                op0=add, op1=amax)

                if img == 0:
                    nc.sync.dma_start(out=w2_sb, in_=w2[:, :, :])

                # ---- layer 2 ----
                for m2 in range(2):
                    ps2 = prps.tile([128, 512], F32, tag="pre")
                    for kc in range(4):
                        nc.tensor.matmul(
                            ps2[:, 0:NPAIR],
                            w2_sb[:, kc, m2 * 128:(m2 + 1) * 128],
                            x1T[:, kc, img * NPAIR:(img + 1) * NPAIR],
                            start=(kc == 0), stop=(kc == 3))
                    dst = x2T[:, m2, img * NPAIR:(img + 1) * NPAIR]
                    if m2 == 0:
                        nc.scalar.activation(dst, ps2[:, 0:NPAIR], relu,
                                             bias=misc_sb[:, 4 + m2:5 + m2])
                    else:
                        nc.vector.tensor_scalar(dst, ps2[:, 0:NPAIR],
                                                misc_sb[:, 4 + m2:5 + m2], 0.0,
                                                op0=add, op1=amax)

                if img == 0:
                    nc.sync.dma_start(out=w3_sb, in_=w3[:, :, :])

                # ---- layer 3 + bias, per-image output store ----
                for h in range(2):
                    s = img * NPAIR + h * 96
                    ps3 = prps.tile([128, 512], F32, tag="pre")
                    for kc in range(2):
                        nc.tensor.matmul(ps3[0:96, 0:128], x2T[:, kc, s:s + 96],
                                         w3_sb[:, kc, :],
                                         start=(kc == 0), stop=(kc == 1))
                    nc.vector.tensor_tensor(
                        out_sb[:, img, h, :], ps3[0:96, 0:128],
                        misc_sb[0:96, 6:134], op=add)
                nc.sync.dma_start(
                    out=out[img * NPAIR:(img + 1) * NPAIR, :]
                    .rearrange("(g p) n -> p g n", p=96),
                    in_=out_sb[:, img])
    nc.compile()
    return nc


def _get_program():
    global _PROGRAM
    if _PROGRAM is None:
        _PROGRAM = _build_program()
    return _PROGRAM


def _box_corners(boxes):
    cx, cy, bw, bh = boxes[..., 0], boxes[..., 1], boxes[..., 2], boxes[..., 3]
    x1 = np.floor((cx - bw / 2) * GRID).astype(np.int64)
    y1 = np.floor((cy - bh / 2) * GRID).astype(np.int64)
    x2 = np.floor((cx + bw / 2) * GRID).astype(np.int64)
    y2 = np.floor((cy + bh / 2) * GRID).astype(np.int64)
    return x1, y1, x2, y2


def _quantize_ef(crop):
    """fp8e4m3 with 2D error feedback over [B, WIN, WIN, C] (carry 1/2 right,
    1/2 down) so rectangle sums of the quantized values track the exact ones."""
    q = np.empty_like(crop)
    nb = crop.shape[0]
    carry_down = np.zeros((nb, WIN, C), np.float32)
    for y in range(WIN):
        carry_x = np.zeros((nb, C), np.float32)
        row_down = np.zeros((nb, WIN, C), np.float32)
        for x in range(WIN):
            v = crop[:, y, x, :] + carry_x + carry_down[:, x, :]
            qv = v.astype(F8).astype(np.float32)
            e = v - qv
            carry_x = 0.5 * e
            row_down[:, x, :] = 0.5 * e
            q[:, y, x, :] = qv
        carry_down = row_down
    return q


def _preprocess(features, boxes, scores):
    """Crop+quantize+pack features; rasterize score-sorted masks.  Returns
    None (-> numpy fallback) if any image's boxes exceed the 52x52 window or
    cover more than KPIX pixels."""
    x1, y1, x2, y2 = _box_corners(boxes)
    r0 = np.clip(y1.min(axis=1), 0, GRID - WIN)
    c0 = np.clip(x1.min(axis=1), 0, GRID - WIN)
    if (y2.max(axis=1) > r0 + WIN).any() or (x2.max(axis=1) > c0 + WIN).any():
        return None

    hidx = np.argsort(-scores[:, :NH], axis=1, kind="stable")
    oidx = np.argsort(-scores[:, NH:], axis=1, kind="stable") + NH
    perm = np.concatenate([hidx, oidx], axis=1)                 # [B, D]
    g = np.arange(WIN)
    rows = ((g[None, None, :] >= (y1 - r0[:, None])[..., None])
            & (g[None, None, :] < (y2 - r0[:, None])[..., None]))
    cols = ((g[None, None, :] >= (x1 - c0[:, None])[..., None])
            & (g[None, None, :] < (x2 - c0[:, None])[..., None]))
    rows = np.take_along_axis(rows, perm[..., None], axis=1)
    cols = np.take_along_axis(cols, perm[..., None], axis=1)
    area = rows.sum(-1) * cols.sum(-1)                          # [B, D]
    mask = (rows[:, :, :, None] & cols[:, :, None, :])          # [B, D, W, W]
    covered = mask.any(axis=1).reshape(B, WIN * WIN)            # [B, W*W]
    if (covered.sum(axis=1) > KPIX).any():
        return None

    crop = np.stack([features[b, r0[b]:r0[b] + WIN, c0[b]:c0[b] + WIN, :]
                     for b in range(B)])
    q = _quantize_ef(np.ascontiguousarray(crop, np.float32))
    q = q.reshape(B, WIN * WIN, C)
    mflat = mask.reshape(B, D, WIN * WIN)

    qp = np.zeros((B, KPIX, C), np.float32)
    mp = np.zeros((B, KPIX, D), bool)
    for b in range(B):
        idx = np.flatnonzero(covered[b])
        qp[b, :len(idx)] = q[b, idx]
        mp[b, :len(idx)] = mflat[b, :, idx]
    # [B, KPIX, C] -> [B, 128, NKP, 2, C]; pixel k = kp*256 + t*128 + p
    ft = qp.reshape(B, NKP, 2, 128, C).transpose(0, 3, 1, 2, 4)
    featA = np.ascontiguousarray(ft[..., 0:512]).astype(F8)
    featB = np.ascontiguousarray(ft[..., 512:768]).astype(F8)
    # [B, KPIX, D] -> [128, B, NKP, 2, D]
    maskq = np.ascontiguousarray(
        mp.reshape(B, NKP, 2, 128, D).transpose(3, 0, 1, 2, 4)
    ).astype(F8)
    return featA, featB, maskq, (1.0 / area).astype(np.float32)


def _run(in_maps, trace=False, **kw):
    nc = _get_program()
    return run_bass_kernel_spmd(nc, in_maps, core_ids=list(range(N_CORES)),
                                trace=trace, **kw)


def _make_in_maps(features, boxes, scores, w1, b1, w2, b2, w3, b3):
    features = np.asarray(features, np.float32)
    pre = _preprocess(features,
                      np.asarray(boxes, np.float32),
                      np.asarray(scores, np.float32))
    if pre is None:
        return None
    featA, featB, maskq, inva = pre
    w1t = np.ascontiguousarray(
        np.asarray(w1, np.float32).reshape(12, 128, H1).transpose(1, 0, 2)
    ).astype(BF)
    w2t = np.ascontiguousarray(
        np.asarray(w2, np.float32).reshape(4, 128, H2).transpose(1, 0, 2)
    ).astype(BF)
    w3p = np.zeros((H2, 128), np.float32)
    w3p[:, :H3] = np.asarray(w3, np.float32)
    w3t = np.ascontiguousarray(
        w3p.reshape(2, 128, 128).transpose(1, 0, 2)).astype(BF)
    selm = np.zeros((64, NPAIR), np.float32)
    for i in range(NH):
        for j in range(NO):
            selm[i, i * NO + j] += 1.0
            selm[32 + j, i * NO + j] += 1.0
    selm = selm.astype(BF)
    b1f = np.asarray(b1, np.float32)
    b2f = np.asarray(b2, np.float32)
    b3f = np.asarray(b3, np.float32)

    in_maps = []
    for c in range(N_CORES):
        s = slice(c * BL, (c + 1) * BL)
        misc = np.zeros((128, MISC_W), np.float32)
        misc[:, 0:4] = b1f.reshape(4, 128).T
        misc[:, 4:6] = b2f.reshape(2, 128).T
        misc[:, 6:6 + H3] = b3f[None, :]
        misc[0:D, 134:136] = inva[s].T
        in_maps.append({
            "featA": np.ascontiguousarray(featA[s]),
            "featB": np.ascontiguousarray(featB[s]),
            "maskq": np.ascontiguousarray(maskq[:, s]),
            "w1": w1t, "w2": w2t, "w3": w3t,
            "sel": selm, "misc": misc,
        })
    return in_maps


def _reference_fallback(features, boxes, scores, w1, b1, w2, b2, w3, b3):
    """Numpy reimplementation, used only if boxes exceed the compiled
    window/coverage caps (cannot happen for inputs from setup_inputs)."""
    x1, y1, x2, y2 = _box_corners(boxes)
    g = np.arange(GRID)
    rows = ((g[None, None, :] >= y1[..., None])
            & (g[None, None, :] < y2[..., None])).astype(np.float32)
    cols = ((g[None, None, :] >= x1[..., None])
            & (g[None, None, :] < x2[..., None])).astype(np.float32)
    s = np.einsum('bdh,bhwc,bdw->bdc', rows,
                  features.astype(np.float32), cols)
    pooled = s / (rows.sum(-1) * cols.sum(-1))[..., None]
    hidx = np.argsort(-scores[:, :NH], axis=1, kind="stable")
    oidx = np.argsort(-scores[:, NH:], axis=1, kind="stable")
    hf = np.take_along_axis(pooled[:, :NH], hidx[..., None], axis=1)
    of = np.take_along_axis(pooled[:, NH:], oidx[..., None], axis=1)
    pairs = np.concatenate(
        [np.broadcast_to(hf[:, :, None, :], (B, NH, NO, C)),
         np.broadcast_to(of[:, None, :, :], (B, NH, NO, C))],
        axis=-1).reshape(B, NPAIR, 2 * C)
    x = np.maximum(pairs @ w1 + b1, 0)
    x = np.maximum(x @ w2 + b2, 0)
    return (x @ w3 + b3).astype(np.float32)


def kernel(features, boxes, scores, w1, b1, w2, b2, w3, b3, labels):
    in_maps = _make_in_maps(features, boxes, scores, w1, b1, w2, b2, w3, b3)
    if in_maps is None:
        return _reference_fallback(
            np.asarray(features, np.float32), np.asarray(boxes, np.float32),
            np.asarray(scores, np.float32), np.asarray(w1, np.float32),
            np.asarray(b1, np.float32), np.asarray(w2, np.float32),
            np.asarray(b2, np.float32), np.asarray(w3, np.float32),
            np.asarray(b3, np.float32))
    res = _run(in_maps, trace=False)
    out = np.concatenate(
        [r["out"].reshape(BL, NPAIR, 128)[:, :, :H3] for r in res.results],
        axis=0)
    return np.ascontiguousarray(out.astype(np.float32))
